# revision 38
# baseline (speedup 1.0000x reference)
"""CTC loss (nn.CTCLoss, mean reduction, zero_infinity) on 8 Trainium2 NeuronCores.

Data-parallel over batch B=128 (16 samples per core). Per core:
  * predicts streams as 16 bf16 tiles [128(8 samples x 16 t-rows), C];
    tile 0 is split into two half-width DMAs so the first bulk Exp starts
    ~2.5us in. One ACT Exp per tile computes exp(x) with free-axis
    accumulation; the per-row sumexp lands directly in a column of a shared
    [128,16] accumulator tile. The ACT Exp stream (~94us) is the kernel's
    critical path.
  * The DP feed is fully decoupled from the bulk Exps: the host gathers the
    160 D-slots (E/skip/validity pre-masked via a -1e5 dead value) of bf16
    LOGITS per (sample, t) into one [128, 8*320] tensor. On chip a single
    small ACT Exp (~2us, scheduled into the DMA stall after tile 0's two
    halves) converts it, and two scatter DMAs lay it out per-sample as
    [16, T*160]. Every DP chunk is ready by ~15us, so the whole CTC DP
    hides inside the bulk-Exp shadow.
  * The CTC forward DP runs in the linear domain on DVE in f32 with a
    single state track: p'[s] = (p[s-2]*skip[s] + p[s-1] + p[s]) * E_t[s],
    2 ops per step: W[s,c] = p[s-2+c] * D_t[3s+c] (one strided multiply),
    then a minor-axis tensor_reduce sums the 3 contributions. Every 8
    steps the row max's reciprocal (f32; its Ln is summed at the end,
    cancelling the rounding exactly) is folded into the multiply via
    scalar_tensor_tensor.
  * Readout: Ln over the sumexp accumulator (one ACT op) -> PE matmul
    with a 0/1 selection matrix sums ln Z_t per sample into PSUM. All ACT
    functions live in the natural_log_exp_and_others table set (patched
    table map), so the kernel never switches activation tables.
Host only builds the gathered-logit/mask tensors from the labels,
shards/pre-tiles/casts the inputs, and averages the 8x16 per-sample
losses.
"""

import os
import sys

import numpy as np
import ml_dtypes

for _p in ("/opt/trn_rl_repo",):
    if _p not in sys.path:
        sys.path.insert(0, _p)

import concourse.bass as bass
import concourse.bacc as bacc
import concourse.mybir as mybir
import concourse.tile as tile
from concourse import bass_utils
from concourse import hw_specs as _hw_specs

F32 = mybir.dt.float32
BF16 = mybir.dt.bfloat16
I16 = mybir.dt.int16

B, T, C, L = 128, 128, 6625, 25
S = 2 * L + 1          # 51 extended-label states
NCORES = 8
BP = B // NCORES       # 16 samples per core
NI3 = 160              # D width per step: 3*51=153 padded to 160
WB = 56                # DP state width (cols 0,1 pad; 2..52 = s)
CSH = 0.58             # host shifts D logits by -CSH, centering the
                       # no-rescale DP's ln-state random walk (worst
                       # |ln state| ~67 on randn inputs) inside bf16's
                       # +-88 exponent window; finalize subtracts T*CSH
TCH = 8                # time chunks
TC = T // TCH          # 16 steps per chunk
BG = 2                 # sample groups per core (tile = 8 samples x 16 t-rows)
BPG = BP // BG         # 8 samples per group
NTL = TCH * BG         # 16 tiles per core
NQ0 = 4                # tile 0 streams as 4 quarter-width DMAs/exps
NOFF = 1               # tiles whose sumexp runs on DVE (Schraudolph 2^y)
CP2 = 6640             # tile width padded so DVE tree-halvings stay
                       # 4B-aligned; pad cols vanish under both exp paths
CHQ = [0, 1657, 3313, 4969, CP2]  # quarter boundaries (even offsets)
PADX = -87.5           # pad logit: exp->~0, Schraudolph int16->denormal~0
SCH_A = np.float32(128.0 / np.log(2.0))


def _calib_schb():
    # Tune the Schraudolph offset so the e^x-weighted mean error of
    # bitcast_bf16(rne_i16(x*SCH_A + B)) vs e^x is zero for x~N(0,1)
    # logits (bf16-quantized, f32 affine, RNE convert - bit-exact vs HW).
    rng = np.random.default_rng(20260810)
    x = rng.standard_normal(2_000_000).astype(ml_dtypes.bfloat16)
    xf = x.astype(np.float32)
    w = np.exp(xf.astype(np.float64))
    b = 16249.0
    for _ in range(3):
        i = np.rint(xf * SCH_A + np.float32(b)).astype(np.int16)
        z = i.view(ml_dtypes.bfloat16).astype(np.float64)
        mu = z.sum() / w.sum() - 1.0
        b -= 128.0 * np.log2(1.0 + mu)
    return np.float32(b)


SCH_B = _calib_schb()

DEAD = -1e5            # dead logit: exp(bf16(DEAD)) == 0

_NC_CACHE = None
last_results = None    # BassKernelResults of the most recent run (for test.py)

_orig_gat = _hw_specs.get_activation_tables


def _gat_single_set(arch):
    # Steer every Exp/Ln to natural_log_exp_and_others so the kernel runs
    # with a single ACT table load and no mid-kernel table switches.
    # Names/order (and therefore act_func_set ids) are preserved.
    t = _orig_gat(arch)
    if "natural_log_exp_and_others" in t:
        for name, fns in t.items():
            if name != "natural_log_exp_and_others":
                fns.discard(mybir.ActivationFunctionType.Exp)
                fns.discard(mybir.ActivationFunctionType.Ln)
    return t


if not os.environ.get("NO_TABLE_PATCH"):
    bacc.get_activation_tables = _gat_single_set


def _ap(base, dims):
    # view with explicit free-axis [stride, num] pairs at base's offset
    return bass.AP(base.tensor, base.offset, [base.ap[0]] + dims)


def _build_nc():
    nc = bacc.Bacc(None, target_bir_lowering=False)
    # pre-tiled on host: tile i=(k*BG+j), row p=b_local*TC+t_sub:
    # xb[i, p, :] = predicts[j*BPG + p//TC, TC*k + p%TC, :]
    xb = nc.dram_tensor("xb", [NTL, 128, CP2], BF16, kind="ExternalInput")
    # host-gathered D logits: row p=(b_local, t_sub), col k*2*NI3+j*NI3+slot
    dl = nc.dram_tensor("dl", [128, TCH * BG * NI3], BF16, kind="ExternalInput")
    initm = nc.dram_tensor("initm", [BP, S], BF16, kind="ExternalInput")
    finalm = nc.dram_tensor("finalm", [BP, S], BF16, kind="ExternalInput")
    w2 = nc.dram_tensor("w2", [128, 2 * BP], F32, kind="ExternalInput")
    lossout = nc.dram_tensor("loss", [BP, 1], F32, kind="ExternalOutput")
    DBG = bool(os.environ.get("BASS_DBG"))
    if DBG:
        smdbg = nc.dram_tensor("smdbg", [128, NTL], F32, kind="ExternalOutput")
        lnrdbg = nc.dram_tensor("lnrdbg", [BP, 2], F32, kind="ExternalOutput")
        ekdbg = nc.dram_tensor("ekdbg", [BP, 2 * NI3], F32, kind="ExternalOutput")

    AX = mybir.AxisListType.X
    AF = mybir.ActivationFunctionType
    OP = mybir.AluOpType

    with tile.TileContext(nc) as tc:
        with (
            tc.tile_pool(name="singles", bufs=1) as singles,
            tc.tile_pool(name="xp", bufs=3) as xp,
            tc.tile_pool(name="etp", bufs=2) as etp,
            tc.tile_pool(name="ekp", bufs=8) as ekp,
            tc.tile_pool(name="xop", bufs=NOFF) as xop,
            tc.tile_pool(name="st", bufs=8) as st,
            tc.tile_pool(name="psp", bufs=1, space="PSUM") as psp,
        ):
            ini = singles.tile([BP, S], BF16, tag="ini")
            nc.scalar.dma_start(out=ini, in_=initm[:, :])
            fin = singles.tile([BP, S], BF16, tag="fin")
            nc.scalar.dma_start(out=fin, in_=finalm[:, :])
            w2s = singles.tile([128, 2 * BP], F32, tag="w2s")
            nc.scalar.dma_start(out=w2s, in_=w2[:, :])

            # DP state: cols 0,1 stay zero (pad), cols 2..52 hold p[s]
            PA = singles.tile([BP, WB], BF16, tag="PA")
            nc.vector.memset(PA, 0.0)
            PB = singles.tile([BP, WB], BF16, tag="PB")
            nc.vector.memset(PB, 0.0)
            Wt = singles.tile([BP, NI3], BF16, tag="Wt")
            SMcol = singles.tile([128, NTL], F32, tag="SMcol")
            SMh = singles.tile([128, NQ0], F32, tag="SMh")

            # Stream ring order: chunks 0-1 of dl first (gates the DP
            # start), tile 0 in quarters (ACT starts at the table-load
            # boundary), tile 1, rest of dl, tiles 2..15.
            NDA = 2 * BG * NI3  # dl columns covering chunks 0-1
            dls = singles.tile([128, TCH * BG * NI3], BF16, tag="dls")
            nc.sync.dma_start(out=dls[:, 0:NDA], in_=dl[:, 0:NDA])
            xt0 = xp.tile([128, CP2], BF16, tag="xt")
            for q in range(NQ0):
                nc.sync.dma_start(
                    out=xt0[:, CHQ[q]:CHQ[q + 1]], in_=xb[0, :, CHQ[q]:CHQ[q + 1]]
                )
            xt1 = xp.tile([128, CP2], BF16, tag="xt")
            nc.sync.dma_start(out=xt1, in_=xb[1, :, :])
            nc.sync.dma_start(out=dls[:, NDA:], in_=dl[:, NDA:])

            # small exps of the gathered D logits (chunks 0-1 first), then
            # scatter each chunk to the per-sample DP layout:
            # ek[k][j*BPG+b, ts*NI3+slot] = es[b*TC+ts, (k*BG+j)*NI3+slot]
            es = singles.tile([128, TCH * BG * NI3], BF16, tag="es")
            eks = []
            for _k in range(TCH):
                ekk = ekp.tile([BP, TC * NI3], BF16, tag="ek")
                eks.append(ekk)

            def scatter_chunk(k):
                for j in range(BG):
                    src = es[:, (k * BG + j) * NI3:(k * BG + j + 1) * NI3]
                    dst = _ap(eks[k][j * BPG:(j + 1) * BPG, 0:1],
                              [[NI3, TC], [1, NI3]])
                    nc.gpsimd.dma_start(out=dst, in_=src)

            nc.scalar.activation(out=es[:, 0:NDA], in_=dls[:, 0:NDA], func=AF.Exp)
            scatter_chunk(0)
            scatter_chunk(1)

            # tile 0 quarters on ACT while tile 1 streams in
            et0 = etp.tile([128, CP2], BF16, tag="et")
            for q in range(NQ0):
                nc.scalar.activation(
                    out=et0[:, CHQ[q]:CHQ[q + 1]], in_=xt0[:, CHQ[q]:CHQ[q + 1]],
                    func=AF.Exp, accum_out=SMh[:, q:q + 1],
                )
            nc.vector.reduce_sum(out=SMcol[:, 0:1], in_=SMh, axis=AX)

            # rest of the gathered-logit exps + scatters
            nc.scalar.activation(out=es[:, NDA:], in_=dls[:, NDA:], func=AF.Exp)
            for k in range(2, TCH):
                scatter_chunk(k)

            # bulk Exp stream, tiles 1..14; tile 15's sumexp runs on DVE
            for i in range(1, NTL - NOFF):
                if i == 1:
                    xt = xt1
                else:
                    xt = xp.tile([128, CP2], BF16, tag="xt")
                    nc.sync.dma_start(out=xt, in_=xb[i, :, :])
                et = etp.tile([128, CP2], BF16, tag="et")
                nc.scalar.activation(
                    out=et, in_=xt, func=AF.Exp,
                    accum_out=SMcol[:, i:i + 1],
                )
            xos = []
            for i in range(NTL - NOFF, NTL):
                xo = xop.tile([128, CP2], BF16, tag="xo")
                nc.sync.dma_start(out=xo, in_=xb[i, :, :])
                xos.append(xo)

            # CTC forward DP (bf16, linear domain). The host's -CSH logit
            # shift keeps the ln-state walk centered; two mid-DP max
            # rescales keep every later Ln input inside the ACT Ln
            # spline's valid range (~e^-46..e^+50).
            RSC = (43, 86)
            RCt = singles.tile([BP, len(RSC)], F32, tag="RCt")
            cur, oth = PA, PB
            pend_rc = None
            with nc.allow_low_precision("ctc linear-domain dp in bf16"):
                for t in range(T):
                    ek = eks[t // TC]
                    tl = t % TC
                    ekb = ek[:, tl * NI3:tl * NI3 + 1]
                    if t == 0:
                        # p0[s] = ini[s] * E_0[s]  (E = D slots 3s+2)
                        nc.vector.tensor_mul(
                            cur[:, 2:2 + S], ini,
                            _ap(ek[:, 2:3], [[3, S]]),
                        )
                    else:
                        # W[s,c] = p[s-2+c] * D_t[3s+c]
                        w_out = _ap(Wt[:, 0:1], [[3, S], [1, 3]])
                        p_in = _ap(cur[:, 0:1], [[1, S], [1, 3]])
                        d_in = _ap(ekb, [[3, S], [1, 3]])
                        if pend_rc is not None:
                            nc.vector.scalar_tensor_tensor(
                                w_out, p_in, pend_rc, d_in, OP.mult, OP.mult,
                            )
                            pend_rc = None
                        else:
                            nc.vector.tensor_mul(w_out, p_in, d_in)
                        # p'[s] = sum_c W[s,c]
                        nc.vector.tensor_reduce(
                            out=oth[:, 2:2 + S],
                            in_=_ap(Wt[:, 0:1], [[3, S], [1, 3]]),
                            axis=AX, op=OP.add,
                        )
                        cur, oth = oth, cur
                    if t in RSC:
                        ksc = RSC.index(t)
                        mx = st.tile([BP, 1], F32, tag="mx")
                        nc.vector.reduce_max(
                            out=mx, in_=cur[:, 2:2 + S], axis=AX
                        )
                        # f32 reciprocal folded into the next multiply; its
                        # Ln is added back at the end, cancelling exactly
                        pend_rc = RCt[:, ksc:ksc + 1]
                        nc.vector.reciprocal(pend_rc, mx)

            lsc = st.tile([BP, len(RSC)], F32, tag="lsc")
            nc.scalar.activation(out=lsc, in_=RCt, func=AF.Ln)
            ssc = st.tile([BP, 1], F32, tag="ssc")
            nc.vector.reduce_sum(out=ssc, in_=lsc, axis=AX)
            wt = singles.tile([BP, S], F32, tag="wt")
            with nc.allow_low_precision("bf16 state readout"):
                nc.vector.tensor_mul(wt, cur[:, 2:2 + S], fin)
            red = st.tile([BP, 1], F32, tag="red")
            nc.vector.reduce_sum(out=red, in_=wt, axis=AX)

            # offloaded sumexp tiles: z = bitcast_bf16(rne_i16(x*A+B)) ~
            # e^x (B host-calibrated to zero the e^x-weighted bias), then
            # bf16 tree-halvings (packed 4x) + a short f32-accum reduce
            ti16 = singles.tile([128, CP2], I16, tag="ti16")
            th1 = singles.tile([128, CP2 // 2], BF16, tag="th1")
            th2 = singles.tile([128, CP2 // 4], BF16, tag="th2")
            th3 = singles.tile([128, CP2 // 8], BF16, tag="th3")
            H1, H2, H3 = CP2 // 2, CP2 // 4, CP2 // 8
            with nc.allow_low_precision("schraudolph sumexp"):
                for n, xo in enumerate(xos):
                    i = NTL - NOFF + n
                    nc.vector.tensor_scalar(
                        out=ti16, in0=xo, scalar1=float(SCH_A),
                        scalar2=float(SCH_B), op0=OP.mult, op1=OP.add,
                    )
                    tb = ti16.bitcast(BF16)
                    nc.vector.tensor_add(th1, tb[:, 0:H1], tb[:, H1:CP2])
                    nc.vector.tensor_add(th2, th1[:, 0:H2], th1[:, H2:H1])
                    nc.vector.tensor_add(th3, th2[:, 0:H3], th2[:, H3:H2])
                    nc.vector.tensor_reduce(
                        out=SMcol[:, i:i + 1], in_=th3, axis=AX, op=OP.add,
                    )
            lnred = st.tile([BP, 1], F32, tag="lnred")
            nc.scalar.activation(out=lnred, in_=red, func=AF.Ln)

            # readout: loss = sum_t ln(sumexp_t) + sum ln(1/scale)
            #                 - ln(sum p_T[final])
            lnsm = singles.tile([128, NTL], F32, tag="lnsm")
            nc.scalar.activation(out=lnsm, in_=SMcol, func=AF.Ln)
            ps = psp.tile([BP, TCH], F32, tag="ps")
            # sum_t ln Z per sample: PSUM[b, k] = sum_j sum_p w2_j[p,b] *
            # lnsm[p, 2k+j]; w2_j[p, b] = 1 iff b == j*8 + p//16
            nc.tensor.matmul(
                ps, w2s[:, 0:BP], _ap(lnsm[:, 0:1], [[2, TCH]]),
                start=True, stop=False,
            )
            nc.tensor.matmul(
                ps, w2s[:, BP:2 * BP], _ap(lnsm[:, 1:2], [[2, TCH]]),
                start=False, stop=True,
            )
            lss = st.tile([BP, 1], F32, tag="lss")
            nc.vector.reduce_sum(out=lss, in_=ps, axis=AX)
            acc2 = st.tile([BP, 1], F32, tag="acc2")
            nc.vector.tensor_add(acc2, lss, ssc)
            ov = st.tile([BP, 1], F32, tag="ov")
            nc.vector.tensor_sub(ov, acc2, lnred)
            nc.scalar.dma_start(out=lossout[:, :], in_=ov)
            if DBG:
                nc.scalar.dma_start(out=smdbg[:, :], in_=SMcol)
                lnr2 = singles.tile([BP, 2], F32, tag="lnr2")
                nc.vector.tensor_copy(out=lnr2[:, 0:1], in_=lnred)
                nc.vector.tensor_copy(out=lnr2[:, 1:2], in_=lss)
                nc.scalar.dma_start(out=lnrdbg[:, :], in_=lnr2)
                ek2 = singles.tile([BP, 2 * NI3], F32, tag="ek2")
                with nc.allow_low_precision("dbg"):
                    nc.vector.tensor_copy(out=ek2[:, 0:NI3], in_=eks[0][:, 0:NI3])
                    nc.vector.tensor_copy(
                        out=ek2[:, NI3:2 * NI3], in_=eks[7][:, (TC - 1) * NI3:]
                    )
                nc.scalar.dma_start(out=ekdbg[:, :], in_=ek2)

    nc.compile()
    return nc


def get_nc():
    global _NC_CACHE
    if _NC_CACHE is None:
        _NC_CACHE = _build_nc()
    return _NC_CACHE


def make_in_maps(predicts, labels, label_lengths):
    predicts = np.asarray(predicts, dtype=np.float32)
    labels = np.asarray(labels)
    lens = np.asarray(label_lengths)
    assert predicts.shape == (B, T, C)

    ext = np.zeros((B, S), np.int64)
    ext[:, 1::2] = labels
    skip = np.zeros((B, S), bool)
    skip[:, 2:] = (ext[:, 2:] != ext[:, :-2])

    initm = np.zeros((B, S), np.float32)
    initm[:, :2] = 1.0
    finalm = np.zeros((B, S), np.float32)
    ar = np.arange(B)
    finalm[ar, 2 * lens] = 1.0
    finalm[ar, 2 * lens - 1] = 1.0

    svec = np.arange(S)
    valid = svec[None, :] <= 2 * lens[:, None]
    # D slots 3s+c: c=2 -> E[s], c=1 -> E[s] (s-1 path), c=0 -> skip-masked
    # E[s] (s-2 path); all dest-validity masked; padding slots dead
    idx3 = np.full((B, NI3), C, np.int64)
    eidx = np.where(valid, ext, C)
    idx3[:, 2:2 + 3 * S:3] = eidx
    idx3[:, 1:1 + 3 * S:3] = eidx
    idx3[:, 0:3 * S:3] = np.where(skip & valid, ext, C)

    # host-gathered D logits: dval[b, t, slot] (dead slots = DEAD),
    # shifted by -CSH so the on-device DP needs no rescaling
    xpad = np.concatenate(
        [predicts, np.full((B, T, 1), DEAD + CSH, np.float32)], axis=2
    )
    dval = (np.take_along_axis(
        xpad, np.broadcast_to(idx3[:, None, :], (B, T, NI3)), axis=2
    ) - CSH).astype(ml_dtypes.bfloat16)

    xb16 = predicts.astype(ml_dtypes.bfloat16)

    # PE selection matrix: w2_j[p, b] = 1 iff b == j*8 + p//16
    w2const = np.zeros((128, 2 * BP), np.float32)
    for j in range(BG):
        for bl in range(BPG):
            w2const[bl * TC:(bl + 1) * TC, j * BP + j * BPG + bl] = 1.0

    in_maps = []
    for cix in range(NCORES):
        b0 = cix * BP
        # pre-tile the shard: [16,T,C] -> [(k j), (b_local t_sub), C+pad]
        xs = xb16[b0:b0 + BP].reshape(BG, BPG, TCH, TC, C)
        xs = xs.transpose(2, 0, 1, 3, 4).reshape(NTL, 128, C)
        xsp = np.full((NTL, 128, CP2), PADX, ml_dtypes.bfloat16)
        xsp[:, :, :C] = xs
        # dl rows (b_local, t_sub), cols (k, j, slot)
        dv = dval[b0:b0 + BP].reshape(BG, BPG, TCH, TC, NI3)
        dv = dv.transpose(1, 3, 2, 0, 4).reshape(128, TCH * BG * NI3)
        in_maps.append({
            "xb": xsp,
            "dl": dv,
            "initm": initm[b0:b0 + BP].astype(ml_dtypes.bfloat16),
            "finalm": finalm[b0:b0 + BP].astype(ml_dtypes.bfloat16),
            "w2": w2const,
        })
    return in_maps


def finalize(loss_raw, label_lengths):
    lens = np.asarray(label_lengths)
    # every one of the T steps multiplied by a e^-CSH-shifted E value
    loss = loss_raw.astype(np.float64) - T * CSH
    loss = np.where(loss > 1e29, 0.0, loss)
    out = (loss / lens.astype(np.float64)).mean() / B
    return np.float32(out)


def kernel(predicts, labels, label_lengths, _trace=False):
    global last_results
    in_maps = make_in_maps(predicts, labels, label_lengths)
    nc = get_nc()
    res = bass_utils.run_bass_kernel_spmd(
        nc, in_maps, core_ids=list(range(NCORES)), trace=_trace
    )
    last_results = res
    loss_raw = np.concatenate([r["loss"][:, 0] for r in res.results])
    return finalize(loss_raw, label_lengths)


# revision 39
# speedup vs baseline: 1.1375x; 1.1375x over previous
"""CTC loss (nn.CTCLoss, mean reduction, zero_infinity) on 8 Trainium2 NeuronCores.

Data-parallel over batch B=128 (16 samples per core). Per core:
  * predicts streams as 16 bf16 tiles [128(8 samples x 16 t-rows), C];
    tile 0 is split into two half-width DMAs so the first bulk Exp starts
    ~2.5us in. One ACT Exp per tile computes exp(x) with free-axis
    accumulation; the per-row sumexp lands directly in a column of a shared
    [128,16] accumulator tile. The ACT Exp stream (~94us) is the kernel's
    critical path.
  * The DP feed is fully decoupled from the bulk Exps: the host gathers the
    160 D-slots (E/skip/validity pre-masked via a -1e5 dead value) of bf16
    LOGITS per (sample, t) into one [128, 8*320] tensor. On chip a single
    small ACT Exp (~2us, scheduled into the DMA stall after tile 0's two
    halves) converts it, and two scatter DMAs lay it out per-sample as
    [16, T*160]. Every DP chunk is ready by ~15us, so the whole CTC DP
    hides inside the bulk-Exp shadow.
  * The CTC forward DP runs in the linear domain on DVE in f32 with a
    single state track: p'[s] = (p[s-2]*skip[s] + p[s-1] + p[s]) * E_t[s],
    2 ops per step: W[s,c] = p[s-2+c] * D_t[3s+c] (one strided multiply),
    then a minor-axis tensor_reduce sums the 3 contributions. Every 8
    steps the row max's reciprocal (f32; its Ln is summed at the end,
    cancelling the rounding exactly) is folded into the multiply via
    scalar_tensor_tensor.
  * Readout: Ln over the sumexp accumulator (one ACT op) -> PE matmul
    with a 0/1 selection matrix sums ln Z_t per sample into PSUM. All ACT
    functions live in the natural_log_exp_and_others table set (patched
    table map), so the kernel never switches activation tables.
Host only builds the gathered-logit/mask tensors from the labels,
shards/pre-tiles/casts the inputs, and averages the 8x16 per-sample
losses.
"""

import os
import sys

import numpy as np
import ml_dtypes

for _p in ("/opt/trn_rl_repo",):
    if _p not in sys.path:
        sys.path.insert(0, _p)

import concourse.bass as bass
import concourse.bacc as bacc
import concourse.mybir as mybir
import concourse.tile as tile
from concourse import bass_utils
from concourse import hw_specs as _hw_specs

F32 = mybir.dt.float32
BF16 = mybir.dt.bfloat16

B, T, C, L = 128, 128, 6625, 25
S = 2 * L + 1          # 51 extended-label states
NCORES = 8
BP = B // NCORES       # 16 samples per core
NI3 = 160              # D width per step: 3*51=153 padded to 160
WB = 56                # DP state width (cols 0,1 pad; 2..52 = s)
CSH = 0.58             # host shifts D logits by -CSH, centering the
                       # no-rescale DP's ln-state random walk (worst
                       # |ln state| ~67 on randn inputs) inside bf16's
                       # +-88 exponent window; finalize subtracts T*CSH
TCH = 8                # time chunks
TC = T // TCH          # 16 steps per chunk
BG = 2                 # sample groups per core (tile = 8 samples x 16 t-rows)
BPG = BP // BG         # 8 samples per group
NTL = TCH * BG         # 16 tiles per core
NQ0 = 4                # tile 0 streams as 4 quarter-width DMAs/exps
CHQ = [0, 1657, 3313, 4969, C]  # quarter boundaries (even offsets)

DEAD = -1e5            # dead logit: exp(bf16(DEAD)) == 0

_NC_CACHE = None
last_results = None    # BassKernelResults of the most recent run (for test.py)

_orig_gat = _hw_specs.get_activation_tables


def _gat_single_set(arch):
    # Steer every Exp/Ln to natural_log_exp_and_others so the kernel runs
    # with a single ACT table load and no mid-kernel table switches.
    # Names/order (and therefore act_func_set ids) are preserved.
    t = _orig_gat(arch)
    if "natural_log_exp_and_others" in t:
        for name, fns in t.items():
            if name != "natural_log_exp_and_others":
                fns.discard(mybir.ActivationFunctionType.Exp)
                fns.discard(mybir.ActivationFunctionType.Ln)
    return t


if not os.environ.get("NO_TABLE_PATCH"):
    bacc.get_activation_tables = _gat_single_set


def _ap(base, dims):
    # view with explicit free-axis [stride, num] pairs at base's offset
    return bass.AP(base.tensor, base.offset, [base.ap[0]] + dims)


def _build_nc():
    nc = bacc.Bacc(None, target_bir_lowering=False)
    # pre-tiled on host: tile i=(k*BG+j), row p=b_local*TC+t_sub:
    # xb[i, p, :] = predicts[j*BPG + p//TC, TC*k + p%TC, :]
    xb = nc.dram_tensor("xb", [NTL, 128, C], BF16, kind="ExternalInput")
    # host-gathered D logits: row p=(b_local, t_sub), col k*2*NI3+j*NI3+slot
    dl = nc.dram_tensor("dl", [128, TCH * BG * NI3], BF16, kind="ExternalInput")
    initm = nc.dram_tensor("initm", [BP, S], BF16, kind="ExternalInput")
    finalm = nc.dram_tensor("finalm", [BP, S], BF16, kind="ExternalInput")
    w2 = nc.dram_tensor("w2", [128, 2 * BP], F32, kind="ExternalInput")
    lossout = nc.dram_tensor("loss", [BP, 1], F32, kind="ExternalOutput")
    DBG = bool(os.environ.get("BASS_DBG"))
    if DBG:
        smdbg = nc.dram_tensor("smdbg", [128, NTL], F32, kind="ExternalOutput")
        lnrdbg = nc.dram_tensor("lnrdbg", [BP, 2], F32, kind="ExternalOutput")
        ekdbg = nc.dram_tensor("ekdbg", [BP, 2 * NI3], F32, kind="ExternalOutput")

    AX = mybir.AxisListType.X
    AF = mybir.ActivationFunctionType
    OP = mybir.AluOpType

    with tile.TileContext(nc) as tc:
        with (
            tc.tile_pool(name="singles", bufs=1) as singles,
            tc.tile_pool(name="xp", bufs=3) as xp,
            tc.tile_pool(name="etp", bufs=2) as etp,
            tc.tile_pool(name="ekp", bufs=8) as ekp,
            tc.tile_pool(name="st", bufs=8) as st,
            tc.tile_pool(name="psp", bufs=1, space="PSUM") as psp,
        ):
            ini = singles.tile([BP, S], BF16, tag="ini")
            nc.scalar.dma_start(out=ini, in_=initm[:, :])
            fin = singles.tile([BP, S], BF16, tag="fin")
            nc.scalar.dma_start(out=fin, in_=finalm[:, :])
            w2s = singles.tile([128, 2 * BP], F32, tag="w2s")
            nc.scalar.dma_start(out=w2s, in_=w2[:, :])

            # DP state: cols 0,1 stay zero (pad), cols 2..52 hold p[s]
            PA = singles.tile([BP, WB], BF16, tag="PA")
            nc.vector.memset(PA, 0.0)
            PB = singles.tile([BP, WB], BF16, tag="PB")
            nc.vector.memset(PB, 0.0)
            Wt = singles.tile([BP, NI3], BF16, tag="Wt")
            SMcol = singles.tile([128, NTL], F32, tag="SMcol")
            SMh = singles.tile([128, NQ0], F32, tag="SMh")

            # Stream ring order: chunks 0-1 of dl first (gates the DP
            # start), tile 0 in quarters (ACT starts at the table-load
            # boundary), tile 1, rest of dl, tiles 2..15.
            NDA = 2 * BG * NI3  # dl columns covering chunks 0-1
            dls = singles.tile([128, TCH * BG * NI3], BF16, tag="dls")
            nc.sync.dma_start(out=dls[:, 0:NDA], in_=dl[:, 0:NDA])
            xt0 = xp.tile([128, C], BF16, tag="xt")
            for q in range(NQ0):
                nc.sync.dma_start(
                    out=xt0[:, CHQ[q]:CHQ[q + 1]], in_=xb[0, :, CHQ[q]:CHQ[q + 1]]
                )
            xt1 = xp.tile([128, C], BF16, tag="xt")
            nc.sync.dma_start(out=xt1, in_=xb[1, :, :])
            nc.sync.dma_start(out=dls[:, NDA:], in_=dl[:, NDA:])

            # small exps of the gathered D logits (chunks 0-1 first), then
            # scatter each chunk to the per-sample DP layout:
            # ek[k][j*BPG+b, ts*NI3+slot] = es[b*TC+ts, (k*BG+j)*NI3+slot]
            es = singles.tile([128, TCH * BG * NI3], BF16, tag="es")
            eks = []
            for _k in range(TCH):
                ekk = ekp.tile([BP, TC * NI3], BF16, tag="ek")
                eks.append(ekk)

            def scatter_chunk(k):
                for j in range(BG):
                    src = es[:, (k * BG + j) * NI3:(k * BG + j + 1) * NI3]
                    dst = _ap(eks[k][j * BPG:(j + 1) * BPG, 0:1],
                              [[NI3, TC], [1, NI3]])
                    nc.gpsimd.dma_start(out=dst, in_=src)

            nc.scalar.activation(out=es[:, 0:NDA], in_=dls[:, 0:NDA], func=AF.Exp)
            scatter_chunk(0)
            scatter_chunk(1)

            # tile 0 quarters on ACT while tile 1 streams in
            et0 = etp.tile([128, C], BF16, tag="et")
            for q in range(NQ0):
                nc.scalar.activation(
                    out=et0[:, CHQ[q]:CHQ[q + 1]], in_=xt0[:, CHQ[q]:CHQ[q + 1]],
                    func=AF.Exp, accum_out=SMh[:, q:q + 1],
                )
            nc.vector.reduce_sum(out=SMcol[:, 0:1], in_=SMh, axis=AX)

            # rest of the gathered-logit exps + scatters
            nc.scalar.activation(out=es[:, NDA:], in_=dls[:, NDA:], func=AF.Exp)
            for k in range(2, TCH):
                scatter_chunk(k)

            # bulk Exp stream, tiles 1..15
            for i in range(1, NTL):
                if i == 1:
                    xt = xt1
                else:
                    xt = xp.tile([128, C], BF16, tag="xt")
                    nc.sync.dma_start(out=xt, in_=xb[i, :, :])
                et = etp.tile([128, C], BF16, tag="et")
                nc.scalar.activation(
                    out=et, in_=xt, func=AF.Exp,
                    accum_out=SMcol[:, i:i + 1],
                )

            # CTC forward DP (bf16, linear domain). The host's -CSH logit
            # shift keeps the ln-state walk centered; two mid-DP max
            # rescales keep every later Ln input inside the ACT Ln
            # spline's valid range (~e^-46..e^+50).
            RSC = (43, 86)
            RCt = singles.tile([BP, len(RSC)], F32, tag="RCt")
            cur, oth = PA, PB
            pend_rc = None
            with nc.allow_low_precision("ctc linear-domain dp in bf16"):
                for t in range(T):
                    ek = eks[t // TC]
                    tl = t % TC
                    ekb = ek[:, tl * NI3:tl * NI3 + 1]
                    if t == 0:
                        # p0[s] = ini[s] * E_0[s]  (E = D slots 3s+2)
                        nc.vector.tensor_mul(
                            cur[:, 2:2 + S], ini,
                            _ap(ek[:, 2:3], [[3, S]]),
                        )
                    else:
                        # W[s,c] = p[s-2+c] * D_t[3s+c]
                        w_out = _ap(Wt[:, 0:1], [[3, S], [1, 3]])
                        p_in = _ap(cur[:, 0:1], [[1, S], [1, 3]])
                        d_in = _ap(ekb, [[3, S], [1, 3]])
                        if pend_rc is not None:
                            nc.vector.scalar_tensor_tensor(
                                w_out, p_in, pend_rc, d_in, OP.mult, OP.mult,
                            )
                            pend_rc = None
                        else:
                            nc.vector.tensor_mul(w_out, p_in, d_in)
                        # p'[s] = sum_c W[s,c]
                        nc.vector.tensor_reduce(
                            out=oth[:, 2:2 + S],
                            in_=_ap(Wt[:, 0:1], [[3, S], [1, 3]]),
                            axis=AX, op=OP.add,
                        )
                        cur, oth = oth, cur
                    if t in RSC:
                        ksc = RSC.index(t)
                        mx = st.tile([BP, 1], F32, tag="mx")
                        nc.vector.reduce_max(
                            out=mx, in_=cur[:, 2:2 + S], axis=AX
                        )
                        # f32 reciprocal folded into the next multiply; its
                        # Ln is added back at the end, cancelling exactly
                        pend_rc = RCt[:, ksc:ksc + 1]
                        nc.vector.reciprocal(pend_rc, mx)

            lsc = st.tile([BP, len(RSC)], F32, tag="lsc")
            nc.scalar.activation(out=lsc, in_=RCt, func=AF.Ln)
            ssc = st.tile([BP, 1], F32, tag="ssc")
            nc.vector.reduce_sum(out=ssc, in_=lsc, axis=AX)
            wt = singles.tile([BP, S], F32, tag="wt")
            with nc.allow_low_precision("bf16 state readout"):
                nc.vector.tensor_mul(wt, cur[:, 2:2 + S], fin)
            red = st.tile([BP, 1], F32, tag="red")
            nc.vector.reduce_sum(out=red, in_=wt, axis=AX)
            lnred = st.tile([BP, 1], F32, tag="lnred")
            nc.scalar.activation(out=lnred, in_=red, func=AF.Ln)

            # readout: loss = sum_t ln(sumexp_t) + sum ln(1/scale)
            #                 - ln(sum p_T[final])
            lnsm = singles.tile([128, NTL], F32, tag="lnsm")
            nc.scalar.activation(out=lnsm, in_=SMcol, func=AF.Ln)
            ps = psp.tile([BP, TCH], F32, tag="ps")
            # sum_t ln Z per sample: PSUM[b, k] = sum_j sum_p w2_j[p,b] *
            # lnsm[p, 2k+j]; w2_j[p, b] = 1 iff b == j*8 + p//16
            nc.tensor.matmul(
                ps, w2s[:, 0:BP], _ap(lnsm[:, 0:1], [[2, TCH]]),
                start=True, stop=False,
            )
            nc.tensor.matmul(
                ps, w2s[:, BP:2 * BP], _ap(lnsm[:, 1:2], [[2, TCH]]),
                start=False, stop=True,
            )
            lss = st.tile([BP, 1], F32, tag="lss")
            nc.vector.reduce_sum(out=lss, in_=ps, axis=AX)
            acc2 = st.tile([BP, 1], F32, tag="acc2")
            nc.vector.tensor_add(acc2, lss, ssc)
            ov = st.tile([BP, 1], F32, tag="ov")
            nc.vector.tensor_sub(ov, acc2, lnred)
            nc.scalar.dma_start(out=lossout[:, :], in_=ov)
            if DBG:
                nc.scalar.dma_start(out=smdbg[:, :], in_=SMcol)
                lnr2 = singles.tile([BP, 2], F32, tag="lnr2")
                nc.vector.tensor_copy(out=lnr2[:, 0:1], in_=lnred)
                nc.vector.tensor_copy(out=lnr2[:, 1:2], in_=lss)
                nc.scalar.dma_start(out=lnrdbg[:, :], in_=lnr2)
                ek2 = singles.tile([BP, 2 * NI3], F32, tag="ek2")
                with nc.allow_low_precision("dbg"):
                    nc.vector.tensor_copy(out=ek2[:, 0:NI3], in_=eks[0][:, 0:NI3])
                    nc.vector.tensor_copy(
                        out=ek2[:, NI3:2 * NI3], in_=eks[7][:, (TC - 1) * NI3:]
                    )
                nc.scalar.dma_start(out=ekdbg[:, :], in_=ek2)

    nc.compile()
    return nc


def get_nc():
    global _NC_CACHE
    if _NC_CACHE is None:
        _NC_CACHE = _build_nc()
    return _NC_CACHE


def make_in_maps(predicts, labels, label_lengths):
    predicts = np.asarray(predicts, dtype=np.float32)
    labels = np.asarray(labels)
    lens = np.asarray(label_lengths)
    assert predicts.shape == (B, T, C)

    ext = np.zeros((B, S), np.int64)
    ext[:, 1::2] = labels
    skip = np.zeros((B, S), bool)
    skip[:, 2:] = (ext[:, 2:] != ext[:, :-2])

    initm = np.zeros((B, S), np.float32)
    initm[:, :2] = 1.0
    finalm = np.zeros((B, S), np.float32)
    ar = np.arange(B)
    finalm[ar, 2 * lens] = 1.0
    finalm[ar, 2 * lens - 1] = 1.0

    svec = np.arange(S)
    valid = svec[None, :] <= 2 * lens[:, None]
    # D slots 3s+c: c=2 -> E[s], c=1 -> E[s] (s-1 path), c=0 -> skip-masked
    # E[s] (s-2 path); all dest-validity masked; padding slots dead
    idx3 = np.full((B, NI3), C, np.int64)
    eidx = np.where(valid, ext, C)
    idx3[:, 2:2 + 3 * S:3] = eidx
    idx3[:, 1:1 + 3 * S:3] = eidx
    idx3[:, 0:3 * S:3] = np.where(skip & valid, ext, C)

    # host-gathered D logits: dval[b, t, slot] (dead slots = DEAD),
    # shifted by -CSH so the on-device DP needs no rescaling
    xpad = np.concatenate(
        [predicts, np.full((B, T, 1), DEAD + CSH, np.float32)], axis=2
    )
    dval = (np.take_along_axis(
        xpad, np.broadcast_to(idx3[:, None, :], (B, T, NI3)), axis=2
    ) - CSH).astype(ml_dtypes.bfloat16)

    xb16 = predicts.astype(ml_dtypes.bfloat16)

    # PE selection matrix: w2_j[p, b] = 1 iff b == j*8 + p//16
    w2const = np.zeros((128, 2 * BP), np.float32)
    for j in range(BG):
        for bl in range(BPG):
            w2const[bl * TC:(bl + 1) * TC, j * BP + j * BPG + bl] = 1.0

    in_maps = []
    for cix in range(NCORES):
        b0 = cix * BP
        # pre-tile the shard: [16,T,C] -> [(k j), (b_local t_sub), C]
        xs = xb16[b0:b0 + BP].reshape(BG, BPG, TCH, TC, C)
        xs = xs.transpose(2, 0, 1, 3, 4).reshape(NTL, 128, C)
        # dl rows (b_local, t_sub), cols (k, j, slot)
        dv = dval[b0:b0 + BP].reshape(BG, BPG, TCH, TC, NI3)
        dv = dv.transpose(1, 3, 2, 0, 4).reshape(128, TCH * BG * NI3)
        in_maps.append({
            "xb": xs,
            "dl": dv,
            "initm": initm[b0:b0 + BP].astype(ml_dtypes.bfloat16),
            "finalm": finalm[b0:b0 + BP].astype(ml_dtypes.bfloat16),
            "w2": w2const,
        })
    return in_maps


def finalize(loss_raw, label_lengths):
    lens = np.asarray(label_lengths)
    # every one of the T steps multiplied by a e^-CSH-shifted E value
    loss = loss_raw.astype(np.float64) - T * CSH
    loss = np.where(loss > 1e29, 0.0, loss)
    out = (loss / lens.astype(np.float64)).mean() / B
    return np.float32(out)


def kernel(predicts, labels, label_lengths, _trace=False):
    global last_results
    in_maps = make_in_maps(predicts, labels, label_lengths)
    nc = get_nc()
    res = bass_utils.run_bass_kernel_spmd(
        nc, in_maps, core_ids=list(range(NCORES)), trace=_trace
    )
    last_results = res
    loss_raw = np.concatenate([r["loss"][:, 0] for r in res.results])
    return finalize(loss_raw, label_lengths)
